# revision 1
# baseline (speedup 1.0000x reference)
import sys
import time
import numpy as np

sys.path.insert(0, "/opt/trn_rl_repo")

import concourse.bass as bass  # noqa: E402
from concourse import mybir  # noqa: E402
from concourse.bass_utils import run_bass_kernel_spmd  # noqa: E402

# nn_GCN_13030930776648: 2-layer RGCN (PyG RGCNConv semantics) on 8 TRN2
# NeuronCores.  Sharding: dst-node ranges (one range per core, no all-reduce).
# Per core and per layer (one NEFF launch per layer):
#   - build per-relation node tables y_r = x @ W_r on device (DVE), stream to
#     a DRAM table [3*2^20, 2] f32
#   - gather one message per edge slot via [P,1] indirect DMA (row y[g],
#     g = rel*2^20 + src), slots laid out in degree-class padded groups
#   - multiply by per-slot weight 1/cnt_rel(dst) (padding slots weight 0)
#   - reduce each node's D-slot group with static strided DVE adds
#   - add root term + bias (+ relu for layer 1)
# Host: structural prep (sort/shard/pad), all-gather of h between the two
# launches, final row assembly.
N_NODES = 1_000_000
NP2 = 1 << 20
NUM_REL = 3
P = 128
NCORES = 8
NC_NODES = NP2 // NCORES
CW = 1024          # slots per msg tile (double buffered)
CHKI = 256         # table-build nodes per partition per chunk

LAST_DEVICE_NS = 0

_f32 = mybir.dt.float32
_i32 = mybir.dt.int32


def _build_layout(src, dst, rel, inv_cnt_w):
    """Host structural prep. Returns per-core [P, L] offset/weight arrays,
    the shared class schedule, and per-core (p, row) -> node maps."""
    order = np.argsort(dst, kind="stable")
    src, dst, rel = src[order], dst[order], rel[order]
    w_edge = inv_cnt_w[rel, dst]
    g_edge = (rel.astype(np.int64) * NP2 + src).astype(np.int32)

    deg = np.bincount(dst, minlength=NP2)
    maxdeg = int(deg.max())
    classes = [c for c in (4, 8, 12, 16, 20, 24, 28, 32, 40, 48, 64, 96, 128,
                           192, 256, 384, 512, 768, 1024) if c <= CW]
    assert maxdeg <= classes[-1], f"max degree {maxdeg} unsupported"
    cls_arr = np.asarray(classes)
    node_cls_idx = np.searchsorted(cls_arr, deg)          # [NP2]

    core_of_node = np.arange(NP2) // NC_NODES
    real = np.arange(NP2) < N_NODES

    # per (core, class): node lists (vectorized): sort nodes by (core, class)
    nodes = np.arange(NP2)[real]
    key = core_of_node[real] * len(classes) + node_cls_idx[real]
    nsort = nodes[np.argsort(key, kind="stable")]
    ksort = key[np.argsort(key, kind="stable")]
    grp_starts = np.searchsorted(ksort, np.arange(NCORES * len(classes) + 1))

    counts = np.diff(grp_starts).reshape(NCORES, len(classes))
    Ms = np.ceil(counts / P).astype(np.int64).max(axis=0)  # equalized per class
    sched = [(classes[ci], int(Ms[ci])) for ci in range(len(classes)) if Ms[ci] > 0]
    L = sum(c * m for c, m in sched)
    R = sum(m for _, m in sched)

    offs = np.zeros((NCORES, P, L), dtype=np.int32)
    wgts = np.zeros((NCORES, P, L), dtype=np.float32)
    rowmap = np.full((NCORES, P, R), -1, dtype=np.int64)

    # per-node placement
    node_p = np.zeros(NP2, dtype=np.int64)
    node_slot0 = np.zeros(NP2, dtype=np.int64)   # slot base within [L]
    node_row = np.zeros(NP2, dtype=np.int64)
    for i in range(NCORES):
        j0 = 0
        r0 = 0
        for (c, m) in sched:
            ci = classes.index(c)
            lo, hi = grp_starts[i * len(classes) + ci], grp_starts[i * len(classes) + ci + 1]
            sel = nsort[lo:hi]
            k = np.arange(len(sel))
            p, q = k % P, k // P
            node_p[sel] = p
            node_slot0[sel] = j0 + q * c
            node_row[sel] = r0 + q
            rowmap[i, p, r0 + q] = sel
            j0 += m * c
            r0 += m
        assert j0 == L and r0 == R

    # per-edge slot assignment (edges sorted by dst; rank within node)
    estart = np.searchsorted(dst, np.arange(NP2 + 1))
    t_edge = np.arange(len(dst)) - estart[dst]
    core_e = dst // NC_NODES
    pe = node_p[dst]
    se = node_slot0[dst] + t_edge
    flat = (core_e * P + pe) * L + se
    offs.reshape(-1)[flat] = g_edge
    wgts.reshape(-1)[flat] = w_edge
    return offs, wgts, rowmap, sched, L, R


def _build_nc(c_in, L, R, sched):
    nc = bass.Bass(target_bir_lowering=False)
    n_tchunks = NP2 // (P * CHKI)
    W6 = 2 * NUM_REL

    xfull = nc.dram_tensor("xfull", [NP2 * c_in], _f32, kind="ExternalInput")
    wcat = nc.dram_tensor("wcat", [P, c_in * W6], _f32, kind="ExternalInput")
    rootb = nc.dram_tensor("rootb", [P, (c_in + 1) * 2], _f32, kind="ExternalInput")
    xroot = nc.dram_tensor("xroot", [P, R * c_in], _f32, kind="ExternalInput")
    offd = nc.dram_tensor("offd", [P, L], _i32, kind="ExternalInput")
    wgtd = nc.dram_tensor("wgtd", [P, L], _f32, kind="ExternalInput")
    ydram = nc.dram_tensor("ydram", [NUM_REL * NP2 * 2], _f32)
    hout = nc.dram_tensor("hout", [P, R * 2], _f32, kind="ExternalOutput")

    off_sb = nc.alloc_sbuf_tensor("off_sb", [P, L], _i32)
    wgt_sb = nc.alloc_sbuf_tensor("wgt_sb", [P, L], _f32)
    msg_sb = nc.alloc_sbuf_tensor("msg_sb", [P, 2 * CW * 2], _f32)
    acc_sb = nc.alloc_sbuf_tensor("acc_sb", [P, R * 2], _f32)
    xr_sb = nc.alloc_sbuf_tensor("xr_sb", [P, R * c_in], _f32)
    wcat_sb = nc.alloc_sbuf_tensor("wcat_sb", [P, c_in * W6], _f32)
    rb_sb = nc.alloc_sbuf_tensor("rb_sb", [P, (c_in + 1) * 2], _f32)
    xt_sb = nc.alloc_sbuf_tensor("xt_sb", [P, CHKI * c_in], _f32)
    yt_sb = nc.alloc_sbuf_tensor("yt_sb", [P, CHKI * W6], _f32)

    ldsem = nc.alloc_semaphore("ldsem")
    tsem = nc.alloc_semaphore("tsem")
    csem = nc.alloc_semaphore("csem")
    ysem = nc.alloc_semaphore("ysem")
    gsem = nc.alloc_semaphore("gsem")
    dsem = nc.alloc_semaphore("dsem")
    osem = nc.alloc_semaphore("osem")

    # input loads (SP/HWDGE)
    nc.sync.dma_start(off_sb[:], offd[:, :]).then_inc(ldsem, 16)
    nc.sync.dma_start(wgt_sb[:], wgtd[:, :]).then_inc(ldsem, 16)
    nc.sync.dma_start(xr_sb[:], xroot[:, :]).then_inc(ldsem, 16)
    nc.sync.dma_start(wcat_sb[:], wcat[:, :]).then_inc(ldsem, 16)
    nc.sync.dma_start(rb_sb[:], rootb[:, :]).then_inc(ldsem, 16)
    n_loads = 5

    nc.vector.memset(acc_sb[:], 0.0)
    nc.vector.wait_ge(ldsem, 16 * n_loads)

    # ---- table build: y[r*NP2+n] = x[n] @ W_r, streamed ----
    xflat = xfull[:]
    yflat = ydram[:]
    mult, add = mybir.AluOpType.mult, mybir.AluOpType.add
    for t in range(n_tchunks):
        base = t * P * CHKI
        src_ap = xflat[base * c_in:(base + P * CHKI) * c_in].rearrange(
            "(p a) -> p a", p=P)
        # x-load waits until DVE finished using xt/yt for chunk t-1 and the
        # y write-out DMAs of chunk t-1 are done (yt reuse)
        if t > 0:
            nc.sync.wait_ge(ysem, 16 * NUM_REL * t)
        nc.sync.dma_start(xt_sb[:], src_ap).then_inc(tsem, 16)
        nc.vector.wait_ge(tsem, 16 * (t + 1))
        xv = xt_sb[:].rearrange("p (i c) -> p i c", c=c_in)
        yv = yt_sb[:].rearrange("p (i o) -> p i o", o=W6)
        for o in range(W6):
            w0 = wcat_sb[:][:, o:o + 1]
            nc.vector.tensor_scalar(yv[:, :, o], xv[:, :, 0], w0, None, mult)
            for cc in range(1, c_in):
                wv = wcat_sb[:][:, cc * W6 + o:cc * W6 + o + 1]
                nc.vector.drain()
                nc.vector.scalar_tensor_tensor(
                    yv[:, :, o], xv[:, :, cc], wv, yv[:, :, o], mult, add)
        nc.vector.drain().then_inc(csem, 1)
        nc.sync.wait_ge(csem, t + 1)
        for r in range(NUM_REL):
            dst_ap = yflat[(r * NP2 + base) * 2:(r * NP2 + base + P * CHKI) * 2] \
                .rearrange("(p a) -> p a", p=P).rearrange("p (i o) -> p i o", o=2)
            yr = yt_sb[:].rearrange("p (i o) -> p i o", o=W6)[:, :, 2 * r:2 * r + 2]
            nc.sync.dma_start(dst_ap, yr).then_inc(ysem, 16)
    total_ydmas = n_tchunks * NUM_REL

    # ---- gather + weighted reduce ----
    nc.gpsimd.wait_ge(ldsem, 16 * n_loads)
    nc.gpsimd.wait_ge(ysem, 16 * total_ydmas)
    ytab = ydram[:].rearrange("(g two) -> g two", two=2)
    msg_flat = msg_sb[:]
    av = acc_sb[:].rearrange("p (r two) -> p r two", two=2)

    j0 = 0
    r0 = 0
    calls = 0
    chunk_id = 0
    for c, m in sched:
        mcap = max(1, CW // c)
        q = 0
        while q < m:
            mm = min(mcap, m - q)
            nslot = mm * c
            buf = chunk_id % 2
            mbase = buf * CW * 2
            if chunk_id >= 1:
                nc.gpsimd.wait_ge(dsem, chunk_id)
            for jj in range(nslot):
                j = j0 + q * c + jj
                nc.gpsimd.indirect_dma_start(
                    out=msg_flat[:, mbase + jj * 2:mbase + (jj + 1) * 2],
                    out_offset=None,
                    in_=ytab,
                    in_offset=bass.IndirectOffsetOnAxis(
                        ap=off_sb[:][:, j:j + 1], axis=0),
                ).then_inc(gsem, 16)
            calls += nslot
            nc.vector.wait_ge(gsem, 16 * calls)
            wv = wgt_sb[:][:, j0 + q * c:j0 + q * c + nslot]
            mv = msg_flat[:, mbase:mbase + nslot * 2].rearrange(
                "p (s two) -> p s two", two=2)
            nc.vector.tensor_tensor(mv[:, :, 0], mv[:, :, 0], wv, mult)
            nc.vector.tensor_tensor(mv[:, :, 1], mv[:, :, 1], wv, mult)
            nc.vector.drain()
            a4 = msg_flat[:, mbase:mbase + nslot * 2].rearrange(
                "p (n s two) -> p n s two", n=mm, two=2)
            cur = c
            while cur > 1:
                half = cur // 2
                nc.vector.tensor_tensor(
                    a4[:, :, 0:half, :], a4[:, :, 0:half, :],
                    a4[:, :, half:2 * half, :], add)
                nc.vector.drain()
                if cur % 2:
                    nc.vector.tensor_tensor(
                        a4[:, :, 0:1, :], a4[:, :, 0:1, :],
                        a4[:, :, cur - 1:cur, :], add)
                    nc.vector.drain()
                cur = half
            nc.vector.tensor_copy(
                av[:, r0 + q:r0 + q + mm, :], a4[:, :, 0, :])
            nc.vector.drain().then_inc(dsem, 1)
            q += mm
            chunk_id += 1
        j0 += m * c
        r0 += m
    n_chunks = chunk_id

    # ---- root term + bias ----
    xrv = xr_sb[:].rearrange("p (r c) -> p r c", c=c_in)
    for o in range(2):
        for cc in range(c_in):
            rv = rb_sb[:][:, cc * 2 + o:cc * 2 + o + 1]
            nc.vector.scalar_tensor_tensor(
                av[:, :, o], xrv[:, :, cc], rv, av[:, :, o], mult, add)
            nc.vector.drain()
        bv = rb_sb[:][:, c_in * 2 + o:c_in * 2 + o + 1]
        nc.vector.tensor_scalar(av[:, :, o], av[:, :, o], bv, None, add)
        nc.vector.drain()
    return nc, acc_sb, hout, osem, dsem, n_chunks


def _finish_nc(nc, acc_sb, hout, osem, relu):
    fsem = nc.alloc_semaphore("fsem")
    if relu:
        nc.vector.tensor_scalar(acc_sb[:], acc_sb[:], 0.0, None,
                                mybir.AluOpType.max)
    nc.vector.drain().then_inc(fsem, 1)
    nc.sync.wait_ge(fsem, 1)
    nc.sync.dma_start(hout[:, :], acc_sb[:]).then_inc(osem, 16)
    nc.sync.wait_ge(osem, 16)
    nc.finalize()


def kernel(x, edge_index, edge_attr, W1, root1, b1, W2, root2, b2):
    global LAST_DEVICE_NS
    LAST_DEVICE_NS = 0
    x = np.asarray(x, dtype=np.float32)
    src = np.asarray(edge_index[0], dtype=np.int64)
    dst = np.asarray(edge_index[1], dtype=np.int64)
    rel = np.asarray(edge_attr, dtype=np.int64)
    W1 = np.asarray(W1, dtype=np.float32)
    root1 = np.asarray(root1, dtype=np.float32)
    b1 = np.asarray(b1, dtype=np.float32)
    W2 = np.asarray(W2, dtype=np.float32)
    root2 = np.asarray(root2, dtype=np.float32)
    b2 = np.asarray(b2, dtype=np.float32)

    cnt = np.zeros((NUM_REL, NP2), dtype=np.int64)
    for r in range(NUM_REL):
        cnt[r] = np.bincount(dst[rel == r], minlength=NP2)
    inv_cnt_w = (1.0 / np.maximum(cnt, 1)).astype(np.float32)
    offs, wgts, rowmap, sched, L, R = _build_layout(src, dst, rel, inv_cnt_w)
    valid = rowmap >= 0

    def launch(xglob, Wl, rootl, bl, xroot_pieces, relu):
        c_in = xglob.shape[1]
        nc, acc_sb, hout, osem, dsem, n_chunks = _build_nc(c_in, L, R, sched)
        _finish_nc(nc, acc_sb, hout, osem, relu)
        xflat = np.ascontiguousarray(xglob.astype(np.float32)).ravel()
        wcat = np.tile(
            np.transpose(Wl, (1, 0, 2)).reshape(1, c_in * NUM_REL * 2),
            (P, 1)).astype(np.float32)
        rootb = np.tile(
            np.concatenate([rootl.reshape(c_in, 2), bl.reshape(1, 2)],
                           axis=0).reshape(1, -1), (P, 1)).astype(np.float32)
        in_maps = []
        for i in range(NCORES):
            in_maps.append({
                "xfull": xflat,
                "wcat": wcat,
                "rootb": rootb,
                "xroot": np.ascontiguousarray(
                    xroot_pieces[i].reshape(P, R * c_in)),
                "offd": offs[i],
                "wgtd": wgts[i],
            })
        global LAST_DEVICE_NS
        t0 = time.perf_counter()
        for attempt in range(3):
            try:
                res = run_bass_kernel_spmd(
                    nc, in_maps, core_ids=list(range(NCORES)))
                break
            except Exception:
                if attempt == 2:
                    raise
                time.sleep(30)
        LAST_DEVICE_NS += int((time.perf_counter() - t0) * 1e9)
        return [r["hout"].reshape(P, R, 2) for r in res.results]

    c1 = x.shape[1]
    xpad = np.zeros((NP2, c1), dtype=np.float32)
    xpad[:N_NODES] = x
    xroot1 = np.zeros((NCORES, P, R, c1), dtype=np.float32)
    for i in range(NCORES):
        xroot1[i][valid[i]] = xpad[rowmap[i][valid[i]]]

    h_pieces = launch(xpad, W1, root1, b1, xroot1, relu=True)

    hglob = np.zeros((NP2, 2), dtype=np.float32)
    for i in range(NCORES):
        hglob[rowmap[i][valid[i]]] = h_pieces[i][valid[i]]

    xroot2 = np.zeros((NCORES, P, R, 2), dtype=np.float32)
    for i in range(NCORES):
        xroot2[i][valid[i]] = hglob[rowmap[i][valid[i]]]

    out_pieces = launch(hglob, W2, root2, b2, xroot2, relu=False)

    out = np.zeros((NP2, 2), dtype=np.float32)
    for i in range(NCORES):
        out[rowmap[i][valid[i]]] = out_pieces[i][valid[i]]
    return out[:N_NODES]

